# revision 2
# baseline (speedup 1.0000x reference)
"""Trainium2 Bass kernel v2 for nn_Distogram — bf16 datapath + masked-j windowing.

Per core (j-major layout: partition p' = rotated j>>3, r = j&7):
  * rows are re-sharded so every core's rows live in ONE batch block; each
    core's same-batch j-window is rotated (host rolls all j-indexed arrays by
    plo) so the window starts at partition 0 and all 8 cores share one SPMD
    program. Cross-batch pairs are masked zeros — never computed; host
    assembles them as zeros.
  * pair build in bf16 (4 rows/iter): t0 = tablewin*scx, pc2 = t0+(left+right),
    pairn = pc2*a8. scx ships ch-expanded so t0 runs in DVE 2x mode. The
    relpos table is a static SBUF sliding window (d = RW-1-il-u+r), no
    per-row table DMA.
  * LN stats decomposed: ssq = 2*(sh + left·rightT); sh ships from host, the
    cross term is a small PE matmul per row-group; Sqrt runs in a stats
    phase before the main loop (one act-table load, no thrash with Gelu).
  * MLP on the j-window only: bf16 transposes -> psum, block-diag W_hidden /
    W_out matmuls, gelu on ACT, psum->bf16 stage copies, bf16 output DMA
    (host upcasts).
"""

import os as _os
_os.environ.setdefault("NEURON_RT_RESET_CORES", "1")

import numpy as np
import ml_dtypes

BF = ml_dtypes.bfloat16
CUTOFF = 32
NBINS = 2 * CUTOFF + 1
LN_EPS = 1e-5
N, D, H, SIZE = 1024, 256, 32, 64
NCORES = 8

_PROGRAM_CACHE = {}


def _layout(batch):
    """Row sharding + j-windows from the sorted batch vector."""
    b0 = int((np.asarray(batch) == 0).sum())
    if 384 <= b0 <= 640:
        n0, n1 = 4, 4
        r0, r1 = -(-b0 // n0), -(-(N - b0) // n1)
        RW = 4 * (-(-max(r0, r1) // 4))
        JP0 = -(-b0 // 8)
        plo1 = b0 // 8
        JP = max(JP0, 128 - plo1)
        JP += JP % 2
        starts, rolls, lens = [], [], []
        for c in range(n0):
            starts.append(min(c * r0, b0 - 1))
            lens.append(max(0, min(r0, b0 - c * r0)))
            rolls.append(0)
        for c in range(n1):
            starts.append(min(b0 + c * r1, N - 1))
            lens.append(max(0, min(r1, N - b0 - c * r1)))
            rolls.append(plo1)
    else:  # degenerate: one batch block -> classic full-width layout
        RW, JP = 128, 128
        starts = [c * 128 for c in range(8)]
        lens = [128] * 8
        rolls = [0] * 8
    return dict(b0=b0, RW=RW, JP=JP, starts=starts, lens=lens, rolls=rolls,
                TABD=RW + 8)


def _build_program(lay, repeat=1, timing=False, cfg=None):
    cfg = dict(dict(rl='dve', pairn='dve', b4sb='act', oc=('act', 'act'),
                    rows=4, scx_exp=True, bigbufs=3, outbufs=3),
               **(cfg or {}))
    import concourse.mybir as mybir
    from concourse import bacc
    from concourse.tile import TileContext
    from concourse.masks import make_identity
    from contextlib import ExitStack
    from bass_rust import AP as RAP

    f32 = mybir.dt.float32
    bf16 = mybir.dt.bfloat16
    AF = mybir.ActivationFunctionType
    OP = mybir.AluOpType
    RW, JP, TABD = lay["RW"], lay["JP"], lay["TABD"]
    GR = RW // 4          # stats group size
    UR = cfg["rows"]      # i-rows per main-loop iteration
    NIT = RW // UR        # main loop iterations
    JW = 8 * JP           # j-window width in elements
    ST = 2 * JW           # stage columns

    nc = bacc.Bacc()
    localT = nc.dram_tensor("localT", [D, N], bf16, kind="ExternalInput")
    myT = nc.dram_tensor("myT", [D, RW], bf16, kind="ExternalInput")
    wlc = nc.dram_tensor("wlc", [D, H], bf16, kind="ExternalInput")
    wrc = nc.dram_tensor("wrc", [D, H], bf16, kind="ExternalInput")
    g0s_d = nc.dram_tensor("g0s", [128, TABD, H], bf16, kind="ExternalInput")
    if cfg["scx_exp"]:
        scx_d = nc.dram_tensor("scx", [128, RW, 8, H], bf16, kind="ExternalInput")
    else:
        scx_d = nc.dram_tensor("scx", [128, RW, 8], bf16, kind="ExternalInput")
    pmd_d = nc.dram_tensor("pmd", [128, 8, RW], bf16, kind="ExternalInput")
    shd_d = nc.dram_tensor("shd", [128, 8, RW], bf16, kind="ExternalInput")
    whg_d = nc.dram_tensor("whg_bd", [128, 128], bf16, kind="ExternalInput")
    wo_d = nc.dram_tensor("wo_bd", [128, 128], bf16, kind="ExternalInput")
    if timing:
        out_t = nc.dram_tensor("out_t", [RW // 4, 128, ST], bf16)
        marker = nc.dram_tensor("marker", [1, 8], f32, kind="ExternalOutput")
    else:
        out_t = nc.dram_tensor("out_t", [RW // 4, 128, ST], bf16,
                               kind="ExternalOutput")

    with TileContext(nc) as tc, ExitStack() as ctx:
        one = ctx.enter_context(tc.tile_pool(name="one", bufs=1))
        sb = ctx.enter_context(tc.tile_pool(name="sb", bufs=3))
        big = ctx.enter_context(tc.tile_pool(name="big", bufs=cfg["bigbufs"]))
        outp = ctx.enter_context(tc.tile_pool(name="outp", bufs=cfg["outbufs"]))
        ps = ctx.enter_context(tc.tile_pool(name="ps", bufs=2, space="PSUM"))
        psb = ctx.enter_context(tc.tile_pool(name="psb", bufs=1, space="PSUM"))
        pso = ctx.enter_context(tc.tile_pool(name="pso", bufs=1, space="PSUM"))

        # ---------------- preamble: statics ----------------
        ident = one.tile([128, 128], bf16, name="ident")
        make_identity(nc, ident)
        eps_col = one.tile([128, 1], f32, name="eps")
        nc.vector.memset(eps_col, LN_EPS)

        g0s = one.tile([128, TABD, H], bf16, name="g0s")
        nc.sync.dma_start(out=g0s, in_=g0s_d[:, :, :])
        if cfg["scx_exp"]:
            scx = one.tile([128, RW, 8, H], bf16, name="scx")
            nc.sync.dma_start(out=scx, in_=scx_d[:, :, :, :])
        else:
            scx = one.tile([128, RW, 8], bf16, name="scx")
            nc.sync.dma_start(out=scx, in_=scx_d[:, :, :])
        pmd = one.tile([128, 8, RW], bf16, name="pmd")
        nc.sync.dma_start(out=pmd, in_=pmd_d[:, :, :])
        shd = one.tile([128, 8, RW], bf16, name="shd")
        nc.sync.dma_start(out=shd, in_=shd_d[:, :, :])
        whg_bd = one.tile([128, 128], bf16, name="whg_bd")
        nc.sync.dma_start(out=whg_bd, in_=whg_d[:, :])
        wo_bd = one.tile([128, 128], bf16, name="wo_bd")
        nc.sync.dma_start(out=wo_bd, in_=wo_d[:, :])

        lt = []
        for k in range(2):
            t = one.tile([128, N], bf16, name=f"lt{k}")
            nc.sync.dma_start(out=t, in_=localT[128 * k:128 * (k + 1), :])
            lt.append(t)
        mt = []
        for k in range(2):
            t = one.tile([128, RW], bf16, name=f"mt{k}")
            nc.sync.dma_start(out=t, in_=myT[128 * k:128 * (k + 1), :])
            mt.append(t)
        wl_s, wr_s = [], []
        for k in range(2):
            t = one.tile([128, H], bf16, name=f"wl{k}")
            nc.sync.dma_start(out=t, in_=wlc[128 * k:128 * (k + 1), :])
            wl_s.append(t)
            t = one.tile([128, H], bf16, name=f"wr{k}")
            nc.sync.dma_start(out=t, in_=wrc[128 * k:128 * (k + 1), :])
            wr_s.append(t)

        # right_s[p, r, ch] = right[8p+r (rotated), ch]
        right_s = one.tile([128, 8, H], bf16, name="right_s")
        for rh in range(2):
            racc = psb.tile([128, 4, H], f32, name="racc", tag="h4")
            for r in range(4):
                for k in range(2):
                    nc.tensor.matmul(racc[:, r, :], lt[k][:, (4 * rh + r)::8],
                                     wr_s[k], start=(k == 0), stop=(k == 1))
            nc.scalar.copy(out=right_s[:, 4 * rh:4 * (rh + 1), :], in_=racc)

        # leftT [ch, il] for the stats matmuls
        leftT = one.tile([H, RW], bf16, name="leftT")
        ltacc = psb.tile([H, RW], f32, name="ltacc", tag="h4")
        for k in range(2):
            nc.tensor.matmul(ltacc, wl_s[k], mt[k], start=(k == 0), stop=(k == 1))
        nc.scalar.copy(out=leftT, in_=ltacc)
        # row-major left rows -> DRAM -> broadcast as lbc[p, il, ch]
        left_dram = nc.dram_tensor("left_scratch", [RW, H], bf16)
        HRW = RW // 2
        for rh in range(2):
            lacc = psb.tile([HRW, H], f32, name="lacc", tag="h4")
            for k in range(2):
                nc.tensor.matmul(lacc, mt[k][:, HRW * rh:HRW * (rh + 1)],
                                 wl_s[k], start=(k == 0), stop=(k == 1))
            lrow = sb.tile([HRW, H], bf16, name="lrow")
            nc.scalar.copy(out=lrow, in_=lacc)
            nc.sync.dma_start(out=left_dram[HRW * rh:HRW * (rh + 1), :], in_=lrow)
        lbc = one.tile([128, RW, H], bf16, name="lbc")
        nc.sync.dma_start(
            out=lbc,
            in_=left_dram[None, :, :].to_broadcast((128, RW, H)))

        rightT = one.tile([H, N], bf16, name="rightT")
        for hh in range(2):
            rtacc = psb.tile([H, N // 2], f32, name="rtacc", tag="h4")
            for k in range(2):
                nc.tensor.matmul(rtacc, wr_s[k], lt[k][:, 512 * hh:512 * (hh + 1)],
                                 start=(k == 0), stop=(k == 1))
            nc.scalar.copy(out=rightT[:, 512 * hh:512 * (hh + 1)], in_=rtacc)

        # ---------------- stats phase ----------------
        # a8d[p, r, il] = pm * rsqrt(ssq/H + eps)
        a8d = one.tile([128, 8, RW], bf16, name="a8d")
        for g in range(4):
            il0 = GR * g
            ssqh = sb.tile([128, 8, GR], f32, name="ssqh")
            for rh in range(2):
                lr_ps = psb.tile([128, 4, GR], f32, name="lr_ps", tag="h4")
                for r in range(4):
                    nc.tensor.matmul(lr_ps[:, r, :], rightT[:, (4 * rh + r)::8],
                                     leftT[:, il0:il0 + GR], start=True, stop=True)
                nc.vector.tensor_tensor(
                    out=ssqh[:, 4 * rh:4 * (rh + 1), :], in0=lr_ps,
                    in1=shd[:, 4 * rh:4 * (rh + 1), il0:il0 + GR], op=OP.add)
            srt = sb.tile([128, 8, GR], f32, name="srt")
            nc.scalar.activation(out=srt, in_=ssqh, func=AF.Sqrt, bias=eps_col,
                                 scale=1.0 / 16.0)
            r8 = sb.tile([128, 8, GR], f32, name="r8")
            nc.vector.reciprocal(r8, srt)
            nc.vector.tensor_tensor(
                out=a8d[:, :, il0:il0 + GR], in0=r8,
                in1=pmd[:, :, il0:il0 + GR], op=OP.mult)

        # ---------------- main loop ----------------
        def main_loop():
            for g16 in range(NIT):
                il0 = UR * g16
                _eng = {"dve": nc.vector, "pool": nc.gpsimd}
                rl = big.tile([128, UR, 8, H], bf16, name="rl")
                _rl_e = cfg["rl"] if cfg["rl"] != "alt" else (
                    "dve" if g16 % 2 == 0 else "pool")
                if _rl_e == "split":
                    hu = UR // 2
                    for (e, u0h) in ((nc.vector, 0), (nc.gpsimd, hu)):
                        e.tensor_tensor(
                            out=rl[:, u0h:u0h + hu, :, :],
                            in0=lbc[:, il0 + u0h:il0 + u0h + hu, None, :]
                                .to_broadcast((128, hu, 8, H)),
                            in1=right_s[:, None, :, :].to_broadcast((128, hu, 8, H)),
                            op=OP.add)
                else:
                    _eng[_rl_e].tensor_tensor(
                        out=rl,
                        in0=lbc[:, il0:il0 + UR, None, :].to_broadcast((128, UR, 8, H)),
                        in1=right_s[:, None, :, :].to_broadcast((128, UR, 8, H)),
                        op=OP.add)
                t0 = big.tile([128, UR, 8, H], bf16, name="t0")
                tab_in = RAP(g0s.tensor, g0s.offset + (RW - 1 - il0) * H,
                             [[TABD * H, 128], [-H, UR], [H, 8], [1, H]])
                if cfg["scx_exp"]:
                    sc_in = scx[:, il0:il0 + UR, :, :]
                else:
                    sc_in = (scx[:, il0:il0 + UR, :][:, :, :, None]
                             .to_broadcast((128, UR, 8, H)))
                nc.vector.tensor_tensor(out=t0, in0=tab_in, in1=sc_in, op=OP.mult)
                pc2 = big.tile([128, UR, 8, H], bf16, name="pc2")
                nc.vector.tensor_tensor(out=pc2, in0=t0, in1=rl, op=OP.add)
                pairn = big.tile([128, UR, 8, H], bf16, name="pairn")
                a8_in = (a8d[:, :, il0:il0 + UR].transpose([0, 2, 1])
                         [:, :, :, None].to_broadcast((128, UR, 8, H)))
                _pn_e = cfg["pairn"] if cfg["pairn"] != "alt" else (
                    "dve" if g16 % 2 == 0 else "pool")
                if _pn_e == "split":
                    hu = UR // 2
                    a8h = (a8d[:, :, il0:il0 + UR].transpose([0, 2, 1])
                           [:, :, :, None])
                    for (e, u0h) in ((nc.vector, 0), (nc.gpsimd, hu)):
                        e.tensor_tensor(
                            out=pairn[:, u0h:u0h + hu, :, :],
                            in0=pc2[:, u0h:u0h + hu, :, :],
                            in1=a8h[:, u0h:u0h + hu, :, :]
                                .to_broadcast((128, hu, 8, H)),
                            op=OP.mult)
                else:
                    _eng[_pn_e].tensor_tensor(out=pairn, in0=pc2, in1=a8_in,
                                              op=OP.mult)

                for sub in range(UR // 4):
                  u0 = 4 * sub
                  stage = outp.tile([128, ST], bf16, name="stage")
                  b4 = ps.tile([128, 8, JP], bf16, name="b4", tag="b4")
                  for u in range(4):
                    for chalf in range(2):
                        nc.tensor.transpose(
                            b4[:, 2 * u + chalf, :],
                            pairn[0:JP, u0 + u, 4 * chalf:4 * (chalf + 1), :],
                            ident[0:JP, 0:JP])
                  b4sb = outp.tile([128, 8, JP], bf16, name="b4sb")
                  if cfg["b4sb"] == "act":
                    nc.scalar.copy(out=b4sb, in_=b4)
                  else:
                    nc.vector.tensor_copy(out=b4sb, in_=b4)
                  # matmul psum writes must stay within one 2KB bank: q-halves
                  HQ = 4 * JP
                  h4 = psb.tile([128, 2, 512], f32, name="h4", tag="h4")
                  for q in range(2):
                    nc.tensor.matmul(
                        h4[:, q, 0:HQ], whg_bd,
                        b4sb.rearrange("p a b -> p (a b)")[:, HQ * q:HQ * (q + 1)],
                        start=True, stop=True)
                  hsb = outp.tile([128, 2, HQ], bf16, name="hsb")
                  nc.scalar.activation(out=hsb, in_=h4[:, :, 0:HQ],
                                       func=AF.Gelu_apprx_tanh)
                  o2a = pso.tile([128, 2, 512], f32, name="o2a", tag="o2a")
                  o2b = pso.tile([128, 2, 512], f32, name="o2b", tag="o2b")
                  for q in range(2):
                    nc.tensor.matmul(o2a[:, q, 0:HQ], wo_bd[0:64, :],
                                     hsb[0:64, q, :], start=True, stop=True)
                    nc.tensor.matmul(o2b[:, q, 0:HQ], wo_bd[64:128, :],
                                     hsb[64:128, q, :], start=True, stop=True)
                  for (sl0, src_t, ei) in ((0, o2a, 0), (JW, o2b, 1)):
                    dst = stage[:, sl0:sl0 + JW].rearrange(
                        "p (q x) -> p q x", q=2)
                    if cfg["oc"][ei] == "act":
                        nc.scalar.copy(out=dst, in_=src_t[:, :, 0:HQ])
                    else:
                        nc.vector.tensor_copy(out=dst, in_=src_t[:, :, 0:HQ])
                  nc.sync.dma_start(out=out_t[(UR * g16) // 4 + sub], in_=stage)

        if repeat == 1:
            main_loop()
        else:
            with tc.For_i(0, repeat, 1):
                main_loop()
        if timing:
            mk = one.tile([1, 8], f32, name="mk")
            nc.vector.memset(mk, 1.0)
            nc.sync.dma_start(out=marker[:, :], in_=mk)

    nc.compile()
    return nc


def _host_prep(inputs, lay=None, scx_exp=True):
    local = np.asarray(inputs["local"], dtype=np.float32)
    resi = np.asarray(inputs["resi"])
    chain = np.asarray(inputs["chain"])
    batch = np.asarray(inputs["batch"])
    mask = np.asarray(inputs["mask"])
    w_left = np.asarray(inputs["W_left"], dtype=np.float32)
    w_right = np.asarray(inputs["W_right"], dtype=np.float32)
    w_relpos = np.asarray(inputs["W_relpos"], dtype=np.float32)
    ln_scale = np.asarray(inputs["ln_scale"], dtype=np.float32)
    ln_offset = np.asarray(inputs["ln_offset"], dtype=np.float32)
    w_hidden = np.asarray(inputs["W_hidden"], dtype=np.float32)
    w_out = np.asarray(inputs["W_out"], dtype=np.float32)

    assert (resi == np.arange(N)).all(), "kernel assumes resi == arange(N)"
    assert not np.any(np.abs(ln_offset) > 0), "kernel assumes ln_offset == 0"

    if lay is None:
        lay = _layout(batch)
    RW, TABD = lay["RW"], lay["TABD"]

    wlc = (w_left - w_left.mean(1, keepdims=True)).astype(BF)
    wrc = (w_right - w_right.mean(1, keepdims=True)).astype(BF)
    wrelc = (w_relpos - w_relpos.mean(1, keepdims=True)).astype(np.float32)
    wrelc_b = wrelc.astype(BF).astype(np.float32)

    local_b = local.astype(BF).astype(np.float32)
    left = local_b @ wlc.astype(np.float32)
    right = local_b @ wrc.astype(np.float32)

    samec = ((chain[:, None] == chain[None, :]) &
             (batch[:, None] == batch[None, :])).astype(np.float32)
    pmask = (mask[:, None] & mask[None, :] &
             (batch[:, None] == batch[None, :])).astype(np.float32)

    # sh = (ssq - 2*left@right.T)/2 assembled without N^2*H work
    d_idx = np.clip(resi[:, None] - resi[None, :], -CUTOFF, CUTOFF) + CUTOFF
    lt_t = left @ wrelc_b.T
    rt_t = right @ wrelc_b.T
    s2 = (wrelc_b ** 2).sum(1)
    sl = (left ** 2).sum(1)
    sr = (right ** 2).sum(1)
    ar = np.arange(N)
    sh_full = (0.5 * (sl[:, None] + sr[None, :]) +
               samec * (lt_t[ar[:, None], d_idx] +
                        rt_t[ar[None, :].repeat(N, 0), d_idx] +
                        0.5 * s2[d_idx]))

    whg = ln_scale[:, None] * w_hidden
    whg_bd = np.zeros((128, 128), np.float32)
    for q in range(4):
        whg_bd[H * q:H * (q + 1), H * q:H * (q + 1)] = whg
    wo_bd = np.zeros((128, 128), np.float32)
    for half in range(2):
        for q in range(2):
            wo_bd[64 * half + H * q:64 * half + H * (q + 1),
                  SIZE * q:SIZE * (q + 1)] = w_out

    in_maps = []
    for c in range(NCORES):
        roll = lay["rolls"][c]
        start = lay["starts"][c]
        nrows = lay["lens"][c]
        rowmap = np.minimum(start + np.arange(RW), start + max(nrows - 1, 0))
        jj = (8 * ((np.arange(128)[:, None] + roll) % 128) +
              np.arange(8)[None, :])            # [p', r] -> original j
        # sliding table: g0s[p', d] = wrelc[clip(rowmap-ish...)]; rows are
        # start..start+RW-1 (pad rows repeat the last row, but the table only
        # depends on start + il + u, so use start + (RW-1) - d + ... directly.
        p = np.arange(128)[:, None]
        dd = np.arange(TABD)[None, :]
        pp = (p + roll) % 128
        g0s = wrelc_b[np.clip(start + (RW - 1) - dd - 8 * pp,
                              -CUTOFF, CUTOFF) + CUTOFF]
        # [p', r, il] masks at (i=rowmap[il], j=jj[p', r])
        sc_c = samec[rowmap[None, None, :], jj[:, :, None]]
        pm_c = pmask[rowmap[None, None, :], jj[:, :, None]]
        sh_c = sh_full[rowmap[None, None, :], jj[:, :, None]]
        if scx_exp:
            scx = np.broadcast_to(
                sc_c.transpose(0, 2, 1)[:, :, :, None], (128, RW, 8, H))
        else:
            scx = sc_c.transpose(0, 2, 1)
        # rolled localT: column p'-block = original j-block
        localT_r = local_b.T[:, jj.reshape(-1)]
        m = dict(
            localT=np.ascontiguousarray(localT_r).astype(BF),
            myT=np.ascontiguousarray(local_b[rowmap].T).astype(BF),
            wlc=wlc, wrc=wrc,
            g0s=np.ascontiguousarray(g0s.astype(BF)),
            scx=np.ascontiguousarray(scx.astype(BF)),
            pmd=np.ascontiguousarray(pm_c.astype(BF)),
            shd=np.ascontiguousarray(sh_c.astype(BF)),
            whg_bd=whg_bd.astype(BF), wo_bd=wo_bd.astype(BF),
        )
        in_maps.append(m)
    return in_maps


def _assemble(results, lay):
    """out_t[g16, 64qq+o, JW*bh + JP*(2u+ch) + p'] ->
       out[rowmap[4g16+u], 8((p'+roll)%128) + 4ch + 2bh + qq, o]"""
    RW, JP = lay["RW"], lay["JP"]
    out = np.zeros((N, N, SIZE), np.float32)
    for c_idx, r in enumerate(results):
        t = np.asarray(r["out_t"]).astype(np.float32)   # [NIT, 128, ST]
        roll = lay["rolls"][c_idx]
        start = lay["starts"][c_idx]
        nrows = lay["lens"][c_idx]
        nit = RW // 4
        t = t.reshape(nit, 2, 64, 2, 4, 2, JP)          # g16,qq,o,bh,u,ch,p'
        t = t.transpose(0, 4, 6, 5, 3, 1, 2)            # g16,u | p',ch,bh,qq | o
        t = t.reshape(RW, JP * 8, SIZE)
        jcols = (8 * ((np.arange(JP)[:, None] + roll) % 128) +
                 np.array([4 * ch + 2 * bh + qq
                           for ch in range(2) for bh in range(2)
                           for qq in range(2)])[None, :]).reshape(-1)
        out[start:start + nrows, jcols] = t[:nrows]
    return out


def kernel(**inputs) -> np.ndarray:
    from concourse.bass_utils import run_bass_kernel_spmd

    lay = _layout(np.asarray(inputs["batch"]))
    in_maps = _host_prep(inputs, lay)
    key = (lay["RW"], lay["JP"], tuple(lay["rolls"]))
    if key not in _PROGRAM_CACHE:
        _PROGRAM_CACHE[key] = _build_program(lay)
    nc = _PROGRAM_CACHE[key]
    res = run_bass_kernel_spmd(nc, in_maps, list(range(NCORES)))
    return _assemble(res.results, lay)
